# revision 1
# baseline (speedup 1.0000x reference)
"""Seq2seq RNN with attention on 8 TRN2 NeuronCores.

Strategy: pure data-parallel over batch. B=32 -> 4 batch elements per core.
Each core runs the full encoder (2-layer tanh RNN), decoder (tanh RNN +
dot-product attention) and the final vocab projection (d=256 -> V=32000)
for its batch shard. Host concatenates the per-core logits along batch.

On-device layout: hidden states kept transposed (d on partitions, batch on
free dim) so the recurrent matmul h@U becomes U.T-chunk matmuls with the
natural U layout as lhsT and no per-step transposes. All TensorEngine
operands are bf16 (enables fast weight load); PSUM accumulation is f32.
Logits are written bf16 and upcast on the host.
"""

import numpy as np

import concourse.bass as bass
import concourse.bacc as bacc
import concourse.tile as tile
from concourse import mybir
from concourse.bass_utils import run_bass_kernel_spmd
from concourse.masks import make_identity

D = 256
V = 32000
T = 128  # T_SRC == T_TGT == 128
B = 32
NCORES = 8
BL = B // NCORES  # 4 batch elements per core
KC = D // 128  # 2 d-chunks of 128
DT = mybir.dt.float32
BF = mybir.dt.bfloat16
NPBF = mybir.dt.np(BF)
AF = mybir.ActivationFunctionType
ALU = mybir.AluOpType
AX = mybir.AxisListType

_CACHE = {}


def _build():
    nc = bacc.Bacc(None)

    u_d = nc.declare_dram_parameter("u", [D, D], BF, isOutput=False)
    cwt_d = nc.declare_dram_parameter("ctx_wt", [D, D], BF, isOutput=False)
    wot_d = nc.declare_dram_parameter("w_out_t", [D, V], BF, isOutput=False)
    een_d = nc.declare_dram_parameter("e_en", [V, D], BF, isOutput=False)
    ede_d = nc.declare_dram_parameter("e_de", [V, D], BF, isOutput=False)
    b1_d = nc.declare_dram_parameter("b1", [128, KC], DT, isOutput=False)
    b2_d = nc.declare_dram_parameter("b2", [128, KC], DT, isOutput=False)
    bd_d = nc.declare_dram_parameter("bd", [128, KC], DT, isOutput=False)
    si_d = nc.declare_dram_parameter("src_idx", [T, BL], mybir.dt.int32, isOutput=False)
    ti_d = nc.declare_dram_parameter("tgt_idx", [T, BL], mybir.dt.int32, isOutput=False)
    out_d = nc.declare_dram_parameter("out", [T * BL, V], BF, isOutput=True)

    with tile.TileContext(nc) as tc:
        with (
            tc.tile_pool(name="persist", bufs=1) as pp,
            tc.tile_pool(name="work", bufs=6) as wp,
        ):
            # ---- persistent SBUF tiles ----
            u_sb = pp.tile([128, KC, D], BF, tag="u")
            cwt_sb = pp.tile([128, KC, D], BF, tag="cwt")
            ident = pp.tile([128, 128], DT, tag="ident")
            identb = pp.tile([128, 128], BF, tag="identb")
            b1_sb = pp.tile([128, KC], DT, tag="b1")
            b2_sb = pp.tile([128, KC], DT, tag="b2")
            bd_sb = pp.tile([128, KC], DT, tag="bd")
            si_sb = pp.tile([T, BL], mybir.dt.int32, tag="si")
            ti_sb = pp.tile([T, BL], mybir.dt.int32, tag="ti")
            madd = pp.tile([BL, T], DT, tag="madd")
            xs = pp.tile([128, KC, BL, T], BF, tag="xs")  # x_src' [d_loc,k,b,t]
            xt = pp.tile([128, KC, BL, T], BF, tag="xt")  # x_tgt'
            hd_all = pp.tile([128, KC, BL, T], BF, tag="hd")  # H' [d_loc,k,b,t]
            ht_all = pp.tile([128, BL, KC, 128], BF, tag="ht")  # H_T [t,b,k,d_loc]
            # encoder state: [d_loc, k, layer*BL]: cols 0:BL = h1, BL:2BL = h2
            hcat = pp.tile([128, KC, 2 * BL], BF, tag="hcat")
            hdec = pp.tile([128, KC, BL], BF, tag="hdec")
            houts = pp.tile([128, KC, T * BL], BF, tag="houts")  # outs' [d,k,t*4+b]

            # ---- load constants ----
            for k in range(KC):
                nc.sync.dma_start(out=u_sb[:, k, :], in_=u_d[k * 128:(k + 1) * 128, :])
                nc.sync.dma_start(out=cwt_sb[:, k, :], in_=cwt_d[k * 128:(k + 1) * 128, :])
            nc.sync.dma_start(out=b1_sb[:, :], in_=b1_d[:, :])
            nc.sync.dma_start(out=b2_sb[:, :], in_=b2_d[:, :])
            nc.sync.dma_start(out=bd_sb[:, :], in_=bd_d[:, :])
            nc.sync.dma_start(out=si_sb[:, :], in_=si_d[:, :])
            nc.sync.dma_start(out=ti_sb[:, :], in_=ti_d[:, :])
            make_identity(nc, ident[:, :])
            nc.vector.tensor_copy(out=identb[:, :], in_=ident[:, :])

            # ---- mask: madd[b, t] = (src==0) * -1e9, built as (T,BL) then PE-transposed
            with tc.tile_pool(name="pst", bufs=2, space="PSUM") as pst:
                mf = wp.tile([T, BL], DT, tag="mf")
                nc.vector.tensor_copy(out=mf[:, :], in_=si_sb[:, :])  # int->f32 cast
                m01 = wp.tile([T, BL], DT, tag="m01")
                nc.vector.tensor_scalar(
                    out=m01[:, :], in0=mf[:, :], scalar1=0.0, scalar2=None,
                    op0=ALU.is_equal,
                )
                mps = pst.tile([BL, T], DT, tag="mps")
                nc.tensor.matmul(out=mps[:, :], lhsT=m01[:, :], rhs=ident[:, :],
                                 start=True, stop=True)
                nc.vector.tensor_scalar(
                    out=madd[:, :], in0=mps[:, :], scalar1=-1e9, scalar2=None,
                    op0=ALU.mult,
                )

                # ---- gather embeddings + transpose to [d_loc, k, b, t] ----
                for (idx_sb, e_d, xdst) in ((si_sb, een_d, xs), (ti_sb, ede_d, xt)):
                    for b in range(BL):
                        xg = wp.tile([T, D], BF, tag="xg")
                        nc.gpsimd.indirect_dma_start(
                            out=xg[:, :],
                            out_offset=None,
                            in_=e_d[:, :],
                            in_offset=bass.IndirectOffsetOnAxis(
                                ap=idx_sb[:, b:b + 1], axis=0),
                        )
                        for k in range(KC):
                            tp = pst.tile([128, 128], DT, tag="tp")
                            nc.tensor.matmul(
                                out=tp[:, :], lhsT=xg[:, k * 128:(k + 1) * 128],
                                rhs=identb[:, :], start=True, stop=True)
                            nc.vector.tensor_copy(out=xdst[:, k, b, :], in_=tp[:, :])

            # ---- encoder: both layers' h@U fused into one matmul per (m,k) ----
            with tc.tile_pool(name="pse", bufs=4, space="PSUM") as pse:
                for t in range(T):
                    if t == 0:
                        for m in range(KC):
                            nc.scalar.activation(
                                out=hcat[:, m, 0:BL], in_=xs[:, m, :, 0],
                                func=AF.Tanh, bias=b1_sb[:, m:m + 1])
                        for m in range(KC):
                            nc.scalar.activation(
                                out=hcat[:, m, BL:2 * BL], in_=hcat[:, m, 0:BL],
                                func=AF.Tanh, bias=b2_sb[:, m:m + 1])
                            nc.gpsimd.tensor_copy(out=hd_all[:, m, :, 0],
                                                  in_=hcat[:, m, BL:2 * BL])
                    else:
                        pls = []
                        for m in range(KC):
                            ps = pse.tile([128, 2 * BL], DT, tag="ps")
                            for k in range(KC):
                                nc.tensor.matmul(
                                    out=ps[:, :],
                                    lhsT=u_sb[:, k, m * 128:(m + 1) * 128],
                                    rhs=hcat[:, k, :],
                                    start=(k == 0), stop=(k == KC - 1))
                            pls.append(ps)
                        for m in range(KC):
                            tmp = wp.tile([128, BL], DT, tag="tmp")
                            nc.vector.tensor_add(out=tmp[:, :], in0=pls[m][:, 0:BL],
                                                 in1=xs[:, m, :, t])
                            nc.scalar.activation(
                                out=hcat[:, m, 0:BL], in_=tmp[:, :], func=AF.Tanh,
                                bias=b1_sb[:, m:m + 1])
                        for m in range(KC):
                            tmp = wp.tile([128, BL], DT, tag="tmp")
                            nc.vector.tensor_add(out=tmp[:, :],
                                                 in0=pls[m][:, BL:2 * BL],
                                                 in1=hcat[:, m, 0:BL])
                            nc.scalar.activation(
                                out=hcat[:, m, BL:2 * BL], in_=tmp[:, :],
                                func=AF.Tanh, bias=b2_sb[:, m:m + 1])
                            nc.gpsimd.tensor_copy(out=hd_all[:, m, :, t],
                                                  in_=hcat[:, m, BL:2 * BL])

            # ---- H' -> H_T transposes ----
            with tc.tile_pool(name="pst2", bufs=4, space="PSUM") as pst2:
                for b in range(BL):
                    for k in range(KC):
                        tp = pst2.tile([128, 128], DT, tag="tp2")
                        nc.tensor.matmul(out=tp[:, :], lhsT=hd_all[:, k, b, :],
                                         rhs=identb[:, :], start=True, stop=True)
                        nc.vector.tensor_copy(out=ht_all[:, b, k, :], in_=tp[:, :])

            # ---- decoder ----
            with (
                tc.tile_pool(name="ps_h", bufs=2, space="PSUM") as ps_h,
                tc.tile_pool(name="ps_a", bufs=1, space="PSUM") as ps_a,
                tc.tile_pool(name="ps_c", bufs=2, space="PSUM") as ps_c,
                tc.tile_pool(name="psS", bufs=2, space="PSUM") as psS,
            ):
                for t in range(T):
                    # h = tanh(h@U + x_t + b): matmuls first (read OLD hdec)
                    phs = []
                    for m in range(KC):
                        ps = ps_h.tile([128, BL], DT, tag="ph")
                        for k in range(KC):
                            prev = hd_all[:, k, :, T - 1] if t == 0 else hdec[:, k, :]
                            nc.tensor.matmul(
                                out=ps[:, :],
                                lhsT=u_sb[:, k, m * 128:(m + 1) * 128],
                                rhs=prev,
                                start=(k == 0), stop=(k == KC - 1))
                        phs.append(ps)
                    for m in range(KC):
                        tmp = wp.tile([128, BL], DT, tag="tmp")
                        nc.vector.tensor_add(out=tmp[:, :], in0=phs[m][:, :],
                                             in1=xt[:, m, :, t])
                        nc.scalar.activation(
                            out=hdec[:, m, :], in_=tmp[:, :], func=AF.Tanh,
                            bias=bd_sb[:, m:m + 1])
                    # scores, transposed: S_T[t, b] = H_b'[:, t] . h_b
                    stp = psS.tile([128, BL], DT, tag="sps")
                    for b in range(BL):
                        for k in range(KC):
                            nc.tensor.matmul(
                                out=stp[:, b:b + 1], lhsT=hd_all[:, k, b, :],
                                rhs=hdec[:, k, b:b + 1],
                                start=(k == 0), stop=(k == KC - 1))
                    st_sb = wp.tile([128, BL], BF, tag="st_sb")
                    nc.vector.tensor_copy(out=st_sb[:, :], in_=stp[:, :])
                    # transpose to (BL, T) for the softmax
                    sps2 = ps_a.tile([BL, T], DT, tag="sps2")
                    nc.tensor.matmul(out=sps2[:, :], lhsT=st_sb[:, :],
                                     rhs=identb[:, :], start=True, stop=True)
                    # masked softmax over t with scale 1/16 folded into exp
                    s_sb = wp.tile([BL, T], DT, tag="s_sb")
                    nc.vector.tensor_add(out=s_sb[:, :], in0=sps2[:, :],
                                         in1=madd[:, :])
                    mx = wp.tile([BL, 1], DT, tag="mx")
                    nc.vector.reduce_max(out=mx[:, :], in_=s_sb[:, :], axis=AX.X)
                    nc.vector.tensor_scalar(
                        out=s_sb[:, :], in0=s_sb[:, :], scalar1=mx[:, :1],
                        scalar2=None, op0=ALU.subtract)
                    ex = wp.tile([BL, T], DT, tag="ex")
                    nc.scalar.activation(out=ex[:, :], in_=s_sb[:, :], func=AF.Exp,
                                         scale=1.0 / 16.0)
                    sm = wp.tile([BL, 1], DT, tag="sm")
                    nc.vector.reduce_sum(out=sm[:, :], in_=ex[:, :], axis=AX.X)
                    rs = wp.tile([BL, 1], DT, tag="rs")
                    nc.vector.reciprocal(out=rs[:, :], in_=sm[:, :])
                    alpha = wp.tile([BL, T], BF, tag="alpha")
                    nc.vector.tensor_scalar(
                        out=alpha[:, :], in0=ex[:, :], scalar1=rs[:, :1],
                        scalar2=None, op0=ALU.mult)
                    # alpha (BL,T) -> alphaT (T,BL)
                    aps = ps_a.tile([128, BL], DT, tag="aps")
                    nc.tensor.matmul(out=aps[:, :], lhsT=alpha[:, :],
                                     rhs=identb[:BL, :BL], start=True, stop=True)
                    a_t = wp.tile([128, BL], BF, tag="a_t")
                    nc.vector.tensor_copy(out=a_t[:, :], in_=aps[:, :])
                    # ctx'[d_chunk m, b] = H_T[:,b,m,:]^T @ alphaT[:,b]
                    ctxs = wp.tile([128, KC, BL], BF, tag="ctxs")
                    for m in range(KC):
                        cps = ps_c.tile([128, BL], DT, tag="cps")
                        for b in range(BL):
                            nc.tensor.matmul(
                                out=cps[:, b:b + 1], lhsT=ht_all[:, b, m, :],
                                rhs=a_t[:, b:b + 1], start=True, stop=True)
                        nc.vector.tensor_copy(out=ctxs[:, m, :], in_=cps[:, :])
                    # out' = h' + ctx_W @ ctx'   -> houts[:, m, t*BL:(t+1)*BL]
                    for m in range(KC):
                        ops_ = ps_h.tile([128, BL], DT, tag="ph")
                        for k in range(KC):
                            nc.tensor.matmul(
                                out=ops_[:, :],
                                lhsT=cwt_sb[:, k, m * 128:(m + 1) * 128],
                                rhs=ctxs[:, k, :],
                                start=(k == 0), stop=(k == KC - 1))
                        nc.vector.tensor_add(
                            out=houts[:, m, t * BL:(t + 1) * BL],
                            in0=ops_[:, :], in1=hdec[:, m, :])

            # ---- final projection: logits = outs @ W_out.T (bf16 out) ----
            n_sizes = []
            n0 = 0
            while n0 < V:
                n_sizes.append((n0, min(512, V - n0)))
                n0 += 512
            with (
                tc.tile_pool(name="psL", bufs=4, space="PSUM") as psL,
                tc.tile_pool(name="wpool", bufs=4) as wpool,
                tc.tile_pool(name="lpool", bufs=4) as lpool,
            ):
                for (n0, nv) in n_sizes:
                    wt = []
                    for k in range(KC):
                        wk = wpool.tile([128, 512], BF, tag="wk")
                        nc.sync.dma_start(out=wk[:, :nv],
                                          in_=wot_d[k * 128:(k + 1) * 128, n0:n0 + nv])
                        wt.append(wk)
                    for mt in range(T * BL // 128):
                        pl = psL.tile([128, 512], DT, tag="pl")
                        for k in range(KC):
                            nc.tensor.matmul(
                                out=pl[:, :nv],
                                lhsT=houts[:, k, mt * 128:(mt + 1) * 128],
                                rhs=wt[k][:, :nv],
                                start=(k == 0), stop=(k == KC - 1))
                        lt = lpool.tile([128, 512], BF, tag="lt")
                        nc.vector.tensor_copy(out=lt[:, :nv], in_=pl[:, :nv])
                        nc.sync.dma_start(
                            out=out_d[mt * 128:(mt + 1) * 128, n0:n0 + nv],
                            in_=lt[:, :nv])
    nc.compile()
    return nc


def _prep_in_maps(U, b_enc1, b_enc2, b_dec, E_en, E_de, ctx_W, W_out_de,
                  src_en, tgt_de_in):
    f32 = np.float32
    U = np.ascontiguousarray(U, f32).astype(NPBF)
    ctx_wt = np.ascontiguousarray(np.asarray(ctx_W, f32).T).astype(NPBF)
    w_out_t = np.ascontiguousarray(np.asarray(W_out_de, f32).T).astype(NPBF)
    E_en = np.ascontiguousarray(E_en, f32).astype(NPBF)
    E_de = np.ascontiguousarray(E_de, f32).astype(NPBF)
    b1 = np.ascontiguousarray(np.asarray(b_enc1, f32).reshape(KC, 128).T)
    b2 = np.ascontiguousarray(np.asarray(b_enc2, f32).reshape(KC, 128).T)
    bd = np.ascontiguousarray(np.asarray(b_dec, f32).reshape(KC, 128).T)
    src = np.asarray(src_en).astype(np.int32)
    tgt = np.asarray(tgt_de_in).astype(np.int32)
    in_maps = []
    for i in range(NCORES):
        b0 = i * BL
        in_maps.append({
            "u": U, "ctx_wt": ctx_wt, "w_out_t": w_out_t,
            "e_en": E_en, "e_de": E_de,
            "b1": b1, "b2": b2, "bd": bd,
            "src_idx": np.ascontiguousarray(src[:, b0:b0 + BL]),
            "tgt_idx": np.ascontiguousarray(tgt[:, b0:b0 + BL]),
        })
    return in_maps


def kernel(U, b_enc1, b_enc2, b_dec, E_en, E_de, ctx_W, W_out_de,
           src_en, tgt_de_in, _trace=False, _raw=False):
    if "nc" not in _CACHE:
        _CACHE["nc"] = _build()
    nc = _CACHE["nc"]
    in_maps = _prep_in_maps(U, b_enc1, b_enc2, b_dec, E_en, E_de, ctx_W,
                            W_out_de, src_en, tgt_de_in)
    res = run_bass_kernel_spmd(nc, in_maps, list(range(NCORES)), trace=_trace)
    if _raw:
        return res
    logits = np.empty((T, B, V), np.float32)
    for i in range(NCORES):
        logits[:, i * BL:(i + 1) * BL, :] = (
            res.results[i]["out"].astype(np.float32).reshape(T, BL, V))
    if _trace:
        return logits, res
    return logits



# revision 5
# speedup vs baseline: 2.1025x; 2.1025x over previous
"""Seq2seq RNN with attention on 8 TRN2 NeuronCores.

Strategy v2:
- Host gathers embeddings (x = E[idx] + bias) and pre-transposes to the
  on-device layout, so the device never touches the 32000-row tables.
- Every core redundantly runs the full-batch (B=32) encoder and decoder
  *recurrences only* -- the per-step critical path is just
  PE(matmuls into PSUM, with x/bias injected via identity/ones matmuls)
  -> ACT(tanh).  No DVE on the critical path.
- Attention is computed *after* the decoder scan, batched over all
  timesteps (it does not feed back into the recurrence).
- The final vocab projection is tensor-parallel over V: core i computes
  columns [i*4000, (i+1)*4000) for the full batch, so each core loads
  only 2 MB of W and writes its 1/8 slice of the logits (bf16).
Output rows are (b, t)-major; the host reassembles (T, B, V) f32.
"""

import numpy as np

import concourse.bass as bass
import concourse.bacc as bacc
import concourse.tile as tile
from concourse import mybir
from concourse.bass_utils import run_bass_kernel_spmd
from concourse.masks import make_identity

D = 256
V = 32000
T = 128  # T_SRC == T_TGT == 128
B = 32
NCORES = 8
VL = V // NCORES  # 4000 vocab cols per core
KC = D // 128  # 2 d-chunks of 128
NVB = 8  # vocab col-blocks per core
VB = VL // NVB  # 500 cols per block (fits one PSUM bank in f32)
DT = mybir.dt.float32
BF = mybir.dt.bfloat16
NPBF = mybir.dt.np(BF)
AF = mybir.ActivationFunctionType
ALU = mybir.AluOpType

_CACHE = {}


def _build():
    nc = bacc.Bacc(None)

    u_d = nc.declare_dram_parameter("u", [D, D], BF, isOutput=False)
    cwt_d = nc.declare_dram_parameter("cwt", [D, D], BF, isOutput=False)
    w_d = nc.declare_dram_parameter("w_slice", [D, VL], BF, isOutput=False)
    xs_d = nc.declare_dram_parameter("xs", [128, KC, B, T], BF, isOutput=False)
    xt_d = nc.declare_dram_parameter("xt", [128, KC, B, T], BF, isOutput=False)
    b2r_d = nc.declare_dram_parameter("b2row", [1, D], BF, isOutput=False)
    b2c_d = nc.declare_dram_parameter("b2col", [128, KC], DT, isOutput=False)
    madd_d = nc.declare_dram_parameter("madd", [1, B * T], BF, isOutput=False)
    out_d = nc.declare_dram_parameter("out", [B * T, VL], BF, isOutput=True)

    with tile.TileContext(nc) as tc:
        with (
            tc.tile_pool(name="persist", bufs=1) as pp,
            tc.tile_pool(name="stage", bufs=3) as sp,
        ):
            # ---- persistent SBUF tiles ----
            u_sb = pp.tile([128, KC, D], BF, tag="u")
            cwt_sb = pp.tile([128, KC, D], BF, tag="cwt")
            w_sb = pp.tile([128, KC, VL], BF, tag="w")
            ident = pp.tile([128, 128], DT, tag="ident")
            identb = pp.tile([128, 128], BF, tag="identb")
            ones_b = pp.tile([1, 128], BF, tag="ones")
            b2r_sb = pp.tile([1, D], BF, tag="b2r")
            b2c_sb = pp.tile([128, KC], DT, tag="b2c")
            madd_sb = pp.tile([1, B * T], BF, tag="madd")
            # x'/h' layouts: [d_lo, k, b, t]
            xs = pp.tile([128, KC, B, T], BF, tag="xs")
            xt = pp.tile([128, KC, B, T], BF, tag="xt")
            hd1 = pp.tile([128, KC, B], BF, tag="hd1")  # enc layer-1 state
            he = pp.tile([128, KC, B, T], BF, tag="he")  # enc H'
            hd = pp.tile([128, KC, B, T], BF, tag="hd")  # dec h'
            het = pp.tile([128, B, KC, 128], BF, tag="het")  # He_T[ts, b, k, d]
            ctx = pp.tile([128, KC, B, T], BF, tag="ctx")  # ctx'
            houts = pp.tile([128, KC, B * T], BF, tag="houts")  # outs'[d,k,(b,t)]

            # ---- load constants ----
            nc.sync.dma_start(out=xs[:, :, :, :], in_=xs_d[:, :, :, :])
            for k in range(KC):
                nc.sync.dma_start(out=u_sb[:, k, :], in_=u_d[k * 128:(k + 1) * 128, :])
                nc.sync.dma_start(out=cwt_sb[:, k, :],
                                  in_=cwt_d[k * 128:(k + 1) * 128, :])
            nc.sync.dma_start(out=b2r_sb[:, :], in_=b2r_d[:, :])
            nc.sync.dma_start(out=b2c_sb[:, :], in_=b2c_d[:, :])
            nc.sync.dma_start(out=madd_sb[:, :], in_=madd_d[:, :])
            nc.sync.dma_start(out=xt[:, :, :, :], in_=xt_d[:, :, :, :])
            # W slice streams in during the encoder scan
            for k in range(KC):
                nc.sync.dma_start(out=w_sb[:, k, :], in_=w_d[k * 128:(k + 1) * 128, :])
            make_identity(nc, ident[:, :])
            nc.vector.tensor_copy(out=identb[:, :], in_=ident[:, :])
            nc.gpsimd.memset(ones_b[:, :], 1.0)

            # =============== encoder scan ===============
            with (
                tc.tile_pool(name="pe1", bufs=2, space="PSUM") as pe1,
                tc.tile_pool(name="pe2", bufs=2, space="PSUM") as pe2,
            ):
                for t in range(T):
                    if t == 0:
                        # h1_0 = tanh(x_0)  (x includes b1)
                        nc.scalar.activation(out=hd1[:, :, :], in_=xs[:, :, :, 0],
                                             func=AF.Tanh)
                        # h2_0 = tanh(h1_0 + b2)
                        for m in range(KC):
                            nc.scalar.activation(out=he[:, m, :, 0],
                                                 in_=hd1[:, m, :], func=AF.Tanh,
                                                 bias=b2c_sb[:, m:m + 1])
                        continue
                    # ---- layer 1: p1 = x_t (+b1, folded) + U.T-chunks @ h1 ----
                    p1 = pe1.tile([128, KC, B], DT, tag="p1")
                    nc.tensor.matmul(out=p1[:, :, :], lhsT=identb[:, :],
                                     rhs=xs[:, :, :, t], start=True, stop=False,
                                     skip_group_check=True)
                    for m in range(KC):
                        for k in range(KC):
                            nc.tensor.matmul(
                                out=p1[:, m, :],
                                lhsT=u_sb[:, k, m * 128:(m + 1) * 128],
                                rhs=hd1[:, k, :],
                                start=False, stop=(k == KC - 1),
                                skip_group_check=True)
                    nc.scalar.activation(out=hd1[:, :, :], in_=p1[:, :, :],
                                         func=AF.Tanh)
                    # ---- layer 2: p2 = b2 + h1_t + U.T-chunks @ h2 ----
                    p2 = pe2.tile([128, KC, B], DT, tag="p2")
                    for m in range(KC):
                        nc.tensor.matmul(out=p2[:, m, :],
                                         lhsT=b2r_sb[:, m * 128:(m + 1) * 128],
                                         rhs=ones_b[:, 0:B],
                                         start=(m == 0), stop=False,
                                         skip_group_check=True)
                    for m in range(KC):
                        for k in range(KC):
                            nc.tensor.matmul(
                                out=p2[:, m, :],
                                lhsT=u_sb[:, k, m * 128:(m + 1) * 128],
                                rhs=he[:, k, :, t - 1],
                                start=False, stop=False,
                                skip_group_check=True)
                    nc.tensor.matmul(out=p2[:, :, :], lhsT=identb[:, :],
                                     rhs=hd1[:, :, :], start=False, stop=True,
                                     skip_group_check=True)
                    nc.scalar.activation(out=he[:, :, :, t], in_=p2[:, :, :],
                                         func=AF.Tanh)

            # =============== decoder scan (+ interleaved He transposes) ======
            with (
                tc.tile_pool(name="pd", bufs=2, space="PSUM") as pd,
                tc.tile_pool(name="pt", bufs=2, space="PSUM") as pt,
            ):
                tp_jobs = [(b, m) for b in range(B) for m in range(KC)]  # 64
                for t in range(T):
                    p = pd.tile([128, KC, B], DT, tag="pdec")
                    nc.tensor.matmul(out=p[:, :, :], lhsT=identb[:, :],
                                     rhs=xt[:, :, :, t], start=True, stop=False,
                                     skip_group_check=True)
                    for m in range(KC):
                        for k in range(KC):
                            prev = (he[:, k, :, T - 1] if t == 0
                                    else hd[:, k, :, t - 1])
                            nc.tensor.matmul(
                                out=p[:, m, :],
                                lhsT=u_sb[:, k, m * 128:(m + 1) * 128],
                                rhs=prev,
                                start=False, stop=(k == KC - 1),
                                skip_group_check=True)
                    nc.scalar.activation(out=hd[:, :, :, t], in_=p[:, :, :],
                                         func=AF.Tanh)
                    # one He_T transpose every other step fills PE idle time;
                    # copies go on DVE only (ACT is on the scan critical path)
                    if t % 2 == 1 and tp_jobs:
                        b, m = tp_jobs.pop()
                        tps = pt.tile([128, 128], BF, tag="tps")
                        nc.tensor.transpose(tps[:, :], he[:, m, b, :], identb[:, :])
                        nc.vector.tensor_copy(out=het[:, b, m, :], in_=tps[:, :])

            # =============== attention (batched over t) ===============
            with (
                tc.tile_pool(name="ps", bufs=2, space="PSUM") as ps,
                tc.tile_pool(name="pa", bufs=2, space="PSUM") as pa,
                tc.tile_pool(name="pc", bufs=2, space="PSUM") as pc,
                tc.tile_pool(name="watt", bufs=4) as watt,
            ):
                for b in range(B):
                    # scores S[tt, ts] = mask + sum_d hd[d, tt] he[d, ts]
                    s_ps = ps.tile([128, 128], DT, tag="sps")
                    nc.tensor.matmul(out=s_ps[:, :], lhsT=ones_b[:, :],
                                     rhs=madd_sb[:, b * T:(b + 1) * T],
                                     start=True, stop=False,
                                     skip_group_check=True)
                    for k in range(KC):
                        nc.tensor.matmul(out=s_ps[:, :], lhsT=hd[:, k, b, :],
                                         rhs=he[:, k, b, :],
                                         start=False, stop=(k == KC - 1),
                                         skip_group_check=True)
                    # softmax over ts (free axis), scale 1/16 inside exp
                    ex = watt.tile([128, 128], BF, tag="ex")
                    sm = watt.tile([128, 1], DT, tag="sm")
                    nc.scalar.activation(out=ex[:, :], in_=s_ps[:, :], func=AF.Exp,
                                         scale=1.0 / 16.0, accum_out=sm[:, :])
                    rs = watt.tile([128, 1], DT, tag="rs")
                    nc.vector.reciprocal(out=rs[:, :], in_=sm[:, :])
                    alpha = watt.tile([128, 128], BF, tag="alpha")
                    nc.vector.tensor_scalar(out=alpha[:, :], in0=ex[:, :],
                                            scalar1=rs[:, :1], scalar2=None,
                                            op0=ALU.mult)
                    # alpha [tt, ts] -> alphaT [ts, tt]
                    a_ps = pa.tile([128, 128], BF, tag="aps")
                    nc.tensor.transpose(a_ps[:, :], alpha[:, :], identb[:, :])
                    a_t = watt.tile([128, 128], BF, tag="at")
                    nc.scalar.copy(out=a_t[:, :], in_=a_ps[:, :])
                    # ctx'[d_m, tt] = He_T[ts, d_m].T @ alphaT[ts, tt]
                    for m in range(KC):
                        c_ps = pc.tile([128, 128], DT, tag="cps")
                        nc.tensor.matmul(out=c_ps[:, :], lhsT=het[:, b, m, :],
                                         rhs=a_t[:, :], start=True, stop=True)
                        if m == 0:
                            nc.vector.tensor_copy(out=ctx[:, m, b, :],
                                                  in_=c_ps[:, :])
                        else:
                            nc.scalar.copy(out=ctx[:, m, b, :], in_=c_ps[:, :])

            # ---- outs = hd + ctx @ ctx_W.T  -> houts[d, k, (b,t)] ----
            with tc.tile_pool(name="po", bufs=2, space="PSUM") as po:
                NBG = 8  # groups of 4 batches -> 512 psum cols
                for m in range(KC):
                    for g in range(NBG):
                        o_ps = po.tile([128, 512], DT, tag="ops")
                        for k in range(KC):
                            nc.tensor.matmul(
                                out=o_ps[:, :],
                                lhsT=cwt_sb[:, k, m * 128:(m + 1) * 128],
                                rhs=ctx[:, k, g * 4:(g + 1) * 4, :],
                                start=(k == 0), stop=(k == KC - 1))
                        nc.vector.tensor_add(
                            out=houts[:, m, g * 512:(g + 1) * 512],
                            in0=o_ps[:, :], in1=hd[:, m, g * 4:(g + 1) * 4, :])

            # =============== vocab projection ===============
            with tc.tile_pool(name="pl", bufs=4, space="PSUM") as pl:
                for b in range(B):
                    stg = sp.tile([128, VL], BF, tag="stg")
                    for vb in range(NVB):
                        l_ps = pl.tile([128, VB], DT, tag="lps")
                        for k in range(KC):
                            nc.tensor.matmul(
                                out=l_ps[:, :],
                                lhsT=houts[:, k, b * T:(b + 1) * T],
                                rhs=w_sb[:, k, vb * VB:(vb + 1) * VB],
                                start=(k == 0), stop=(k == KC - 1))
                        if vb % 2 == 0:
                            nc.vector.tensor_copy(
                                out=stg[:, vb * VB:(vb + 1) * VB], in_=l_ps[:, :])
                        else:
                            nc.scalar.copy(
                                out=stg[:, vb * VB:(vb + 1) * VB], in_=l_ps[:, :])
                    nc.sync.dma_start(out=out_d[b * T:(b + 1) * T, :],
                                      in_=stg[:, :])
    nc.compile()
    return nc


def _prep_in_maps(U, b_enc1, b_enc2, b_dec, E_en, E_de, ctx_W, W_out_de,
                  src_en, tgt_de_in):
    f32 = np.float32
    U = np.ascontiguousarray(U, f32).astype(NPBF)
    cwt = np.ascontiguousarray(np.asarray(ctx_W, f32).T).astype(NPBF)
    w_t = np.ascontiguousarray(np.asarray(W_out_de, f32).T)  # [D, V] f32
    E_en = np.asarray(E_en, f32)
    E_de = np.asarray(E_de, f32)
    src = np.asarray(src_en)
    tgt = np.asarray(tgt_de_in)

    def gather_x(E, idx, bias):
        # x'[p, k, b, t] = E[idx[t, b], k*128 + p] + bias[k*128+p]
        x = E[idx] + np.asarray(bias, f32)  # (T, B, D)
        x = x.transpose(2, 1, 0).reshape(KC, 128, B, T)  # (k, p, b, t)
        x = x.transpose(1, 0, 2, 3)  # (p, k, b, t)
        return np.ascontiguousarray(x).astype(NPBF)

    xs = gather_x(E_en, src, b_enc1)
    xt = gather_x(E_de, tgt, b_dec)
    b2row = np.asarray(b_enc2, f32).reshape(1, D).astype(NPBF)
    b2col = np.ascontiguousarray(np.asarray(b_enc2, f32).reshape(KC, 128).T)
    madd = np.where(src == 0, f32(-1e9), f32(0.0)).T.reshape(1, B * T)  # (b,t)
    madd = madd.astype(NPBF)

    in_maps = []
    for i in range(NCORES):
        in_maps.append({
            "u": U, "cwt": cwt,
            "w_slice": np.ascontiguousarray(
                w_t[:, i * VL:(i + 1) * VL]).astype(NPBF),
            "xs": xs, "xt": xt,
            "b2row": b2row, "b2col": b2col, "madd": madd,
        })
    return in_maps


def kernel(U, b_enc1, b_enc2, b_dec, E_en, E_de, ctx_W, W_out_de,
           src_en, tgt_de_in, _trace=False, _raw=False):
    if "nc" not in _CACHE:
        _CACHE["nc"] = _build()
    nc = _CACHE["nc"]
    in_maps = _prep_in_maps(U, b_enc1, b_enc2, b_dec, E_en, E_de, ctx_W,
                            W_out_de, src_en, tgt_de_in)
    res = run_bass_kernel_spmd(nc, in_maps, list(range(NCORES)), trace=_trace)
    if _raw:
        return res
    logits = np.empty((T, B, V), np.float32)
    for i in range(NCORES):
        blk = res.results[i]["out"].astype(np.float32).reshape(B, T, VL)
        logits[:, :, i * VL:(i + 1) * VL] = blk.transpose(1, 0, 2)
    if _trace:
        return logits, res
    return logits


# revision 6
# speedup vs baseline: 2.9703x; 1.4127x over previous
"""Seq2seq RNN with attention on 8 TRN2 NeuronCores.

Strategy v2:
- Host gathers embeddings (x = E[idx] + bias) and pre-transposes to the
  on-device layout, so the device never touches the 32000-row tables.
- Every core redundantly runs the full-batch (B=32) encoder and decoder
  *recurrences only* -- the per-step critical path is just
  PE(matmuls into PSUM, with x/bias injected via identity/ones matmuls)
  -> ACT(tanh).  No DVE on the critical path.
- Attention is computed *after* the decoder scan, batched over all
  timesteps (it does not feed back into the recurrence).
- The final vocab projection is tensor-parallel over V: core i computes
  columns [i*4000, (i+1)*4000) for the full batch, so each core loads
  only 2 MB of W and writes its 1/8 slice of the logits (bf16).
Output rows are (b, t)-major; the host reassembles (T, B, V) f32.
"""

import numpy as np

import concourse.bass as bass
import concourse.bacc as bacc
import concourse.tile as tile
from concourse import mybir
from concourse.bass_utils import run_bass_kernel_spmd
from concourse.masks import make_identity

D = 256
V = 32000
T = 128  # T_SRC == T_TGT == 128
B = 32
NCORES = 8
VL = V // NCORES  # 4000 vocab cols per core
KC = D // 128  # 2 d-chunks of 128
NVB = 8  # vocab col-blocks per core
VB = VL // NVB  # 500 cols per block (fits one PSUM bank in f32)
DT = mybir.dt.float32
BF = mybir.dt.bfloat16
NPBF = mybir.dt.np(BF)
AF = mybir.ActivationFunctionType
ALU = mybir.AluOpType

_CACHE = {}


def _build():
    nc = bacc.Bacc(None)

    u_d = nc.declare_dram_parameter("u", [D, D], BF, isOutput=False)
    cwt_d = nc.declare_dram_parameter("cwt", [D, D], BF, isOutput=False)
    w_d = nc.declare_dram_parameter("w_slice", [D, VL], BF, isOutput=False)
    xs_d = nc.declare_dram_parameter("xs", [128, T, KC, B], BF, isOutput=False)
    xt_d = nc.declare_dram_parameter("xt", [128, T, KC, B], BF, isOutput=False)
    b2r_d = nc.declare_dram_parameter("b2row", [1, D], BF, isOutput=False)
    b2c_d = nc.declare_dram_parameter("b2col", [128, KC], DT, isOutput=False)
    madd_d = nc.declare_dram_parameter("madd", [1, B * T], BF, isOutput=False)
    out_d = nc.declare_dram_parameter("out", [B * T, VL], BF, isOutput=True)

    with tile.TileContext(nc) as tc:
        with (
            tc.tile_pool(name="persist", bufs=1) as pp,
            tc.tile_pool(name="stage", bufs=3) as sp,
        ):
            # ---- persistent SBUF tiles ----
            u_sb = pp.tile([128, KC, D], BF, tag="u")
            cwt_sb = pp.tile([128, KC, D], BF, tag="cwt")
            w_sb = pp.tile([128, KC, VL], BF, tag="w")
            ident = pp.tile([128, 128], DT, tag="ident")
            identb = pp.tile([128, 128], BF, tag="identb")
            ones_b = pp.tile([1, 128], BF, tag="ones")
            b2r_sb = pp.tile([1, D], BF, tag="b2r")
            b2c_sb = pp.tile([128, KC], DT, tag="b2c")
            madd_sb = pp.tile([1, B * T], BF, tag="madd")
            # x'/h' layouts: [d_lo, k, b, t]
            xs = pp.tile([128, T, KC, B], BF, tag="xs")
            xt = pp.tile([128, T, KC, B], BF, tag="xt")
            hd2 = pp.tile([128, 2, KC, B], BF, tag="hd2")  # enc l2 state (dbl-buf)
            hdc = pp.tile([128, 2, KC, B], BF, tag="hdc")  # dec state (dbl-buf)
            hd1 = pp.tile([128, KC, B], BF, tag="hd1")  # enc layer-1 state
            he = pp.tile([128, KC, B, T], BF, tag="he")  # enc H'
            hd = pp.tile([128, KC, B, T], BF, tag="hd")  # dec h'
            het = pp.tile([128, B, KC, 128], BF, tag="het")  # He_T[ts, b, k, d]
            ctx = pp.tile([128, KC, B, T], BF, tag="ctx")  # ctx'
            houts = pp.tile([128, KC, B * T], BF, tag="houts")  # outs'[d,k,(b,t)]

            # ---- load constants ----
            nc.sync.dma_start(out=xs[:, :, :, :], in_=xs_d[:, :, :, :])
            for k in range(KC):
                nc.sync.dma_start(out=u_sb[:, k, :], in_=u_d[k * 128:(k + 1) * 128, :])
                nc.sync.dma_start(out=cwt_sb[:, k, :],
                                  in_=cwt_d[k * 128:(k + 1) * 128, :])
            nc.sync.dma_start(out=b2r_sb[:, :], in_=b2r_d[:, :])
            nc.sync.dma_start(out=b2c_sb[:, :], in_=b2c_d[:, :])
            nc.sync.dma_start(out=madd_sb[:, :], in_=madd_d[:, :])
            nc.sync.dma_start(out=xt[:, :, :, :], in_=xt_d[:, :, :, :])
            # W slice streams in during the encoder scan
            for k in range(KC):
                nc.sync.dma_start(out=w_sb[:, k, :], in_=w_d[k * 128:(k + 1) * 128, :])
            make_identity(nc, ident[:, :])
            nc.vector.tensor_copy(out=identb[:, :], in_=ident[:, :])
            nc.gpsimd.memset(ones_b[:, :], 1.0)

            # =============== encoder scan ===============
            with (
                tc.tile_pool(name="pe1", bufs=2, space="PSUM") as pe1,
                tc.tile_pool(name="pe2", bufs=2, space="PSUM") as pe2,
            ):
                for t in range(T):
                    if t == 0:
                        # h1_0 = tanh(x_0)  (x includes b1)
                        nc.scalar.activation(out=hd1[:, :, :], in_=xs[:, 0, :, :],
                                             func=AF.Tanh)
                        # h2_0 = tanh(h1_0 + b2)
                        for m in range(KC):
                            nc.scalar.activation(out=hd2[:, 0, m, :],
                                                 in_=hd1[:, m, :], func=AF.Tanh,
                                                 bias=b2c_sb[:, m:m + 1])
                        nc.gpsimd.tensor_copy(out=he[:, :, :, 0],
                                              in_=hd2[:, 0, :, :])
                        continue
                    # ---- layer 1: p1 = x_t (+b1, folded) + U.T-chunks @ h1 ----
                    p1 = pe1.tile([128, KC, B], DT, tag="p1")
                    nc.tensor.matmul(out=p1[:, :, :], lhsT=identb[:, :],
                                     rhs=xs[:, t, :, :], start=True, stop=False,
                                     skip_group_check=True)
                    for m in range(KC):
                        for k in range(KC):
                            nc.tensor.matmul(
                                out=p1[:, m, :],
                                lhsT=u_sb[:, k, m * 128:(m + 1) * 128],
                                rhs=hd1[:, k, :],
                                start=False, stop=(k == KC - 1),
                                skip_group_check=True)
                    nc.scalar.activation(out=hd1[:, :, :], in_=p1[:, :, :],
                                         func=AF.Tanh)
                    # ---- layer 2: p2 = b2 + h1_t + U.T-chunks @ h2 ----
                    p2 = pe2.tile([128, KC, B], DT, tag="p2")
                    for m in range(KC):
                        nc.tensor.matmul(out=p2[:, m, :],
                                         lhsT=b2r_sb[:, m * 128:(m + 1) * 128],
                                         rhs=ones_b[:, 0:B],
                                         start=(m == 0), stop=False,
                                         skip_group_check=True)
                    for m in range(KC):
                        for k in range(KC):
                            nc.tensor.matmul(
                                out=p2[:, m, :],
                                lhsT=u_sb[:, k, m * 128:(m + 1) * 128],
                                rhs=hd2[:, (t - 1) % 2, k, :],
                                start=False, stop=False,
                                skip_group_check=True)
                    nc.tensor.matmul(out=p2[:, :, :], lhsT=identb[:, :],
                                     rhs=hd1[:, :, :], start=False, stop=True,
                                     skip_group_check=True)
                    nc.scalar.activation(out=hd2[:, t % 2, :, :],
                                         in_=p2[:, :, :], func=AF.Tanh)
                    nc.gpsimd.tensor_copy(out=he[:, :, :, t],
                                          in_=hd2[:, t % 2, :, :])

            # =============== decoder scan (+ interleaved He transposes) ======
            with (
                tc.tile_pool(name="pd", bufs=2, space="PSUM") as pd,
                tc.tile_pool(name="pt", bufs=2, space="PSUM") as pt,
            ):
                tp_jobs = [(b, m) for b in range(B) for m in range(KC)]  # 64
                for t in range(T):
                    p = pd.tile([128, KC, B], DT, tag="pdec")
                    nc.tensor.matmul(out=p[:, :, :], lhsT=identb[:, :],
                                     rhs=xt[:, t, :, :], start=True, stop=False,
                                     skip_group_check=True)
                    for m in range(KC):
                        for k in range(KC):
                            prev = (hd2[:, (T - 1) % 2, k, :] if t == 0
                                    else hdc[:, (t - 1) % 2, k, :])
                            nc.tensor.matmul(
                                out=p[:, m, :],
                                lhsT=u_sb[:, k, m * 128:(m + 1) * 128],
                                rhs=prev,
                                start=False, stop=(k == KC - 1),
                                skip_group_check=True)
                    nc.scalar.activation(out=hdc[:, t % 2, :, :],
                                         in_=p[:, :, :], func=AF.Tanh)
                    nc.gpsimd.tensor_copy(out=hd[:, :, :, t],
                                          in_=hdc[:, t % 2, :, :])
                    # one He_T transpose every other step fills PE idle time;
                    # copies go on DVE only (ACT is on the scan critical path)
                    if t % 2 == 1 and tp_jobs:
                        b, m = tp_jobs.pop()
                        tps = pt.tile([128, 128], BF, tag="tps")
                        nc.tensor.transpose(tps[:, :], he[:, m, b, :], identb[:, :])
                        nc.vector.tensor_copy(out=het[:, b, m, :], in_=tps[:, :])

            # =============== attention (batched over t) ===============
            with (
                tc.tile_pool(name="ps", bufs=2, space="PSUM") as ps,
                tc.tile_pool(name="pa", bufs=2, space="PSUM") as pa,
                tc.tile_pool(name="pc", bufs=2, space="PSUM") as pc,
                tc.tile_pool(name="watt", bufs=4) as watt,
            ):
                for b in range(B):
                    # scores S[tt, ts] = mask + sum_d hd[d, tt] he[d, ts]
                    s_ps = ps.tile([128, 128], DT, tag="sps")
                    nc.tensor.matmul(out=s_ps[:, :], lhsT=ones_b[:, :],
                                     rhs=madd_sb[:, b * T:(b + 1) * T],
                                     start=True, stop=False,
                                     skip_group_check=True)
                    for k in range(KC):
                        nc.tensor.matmul(out=s_ps[:, :], lhsT=hd[:, k, b, :],
                                         rhs=he[:, k, b, :],
                                         start=False, stop=(k == KC - 1),
                                         skip_group_check=True)
                    # softmax over ts (free axis), scale 1/16 inside exp
                    ex = watt.tile([128, 128], BF, tag="ex")
                    sm = watt.tile([128, 1], DT, tag="sm")
                    nc.scalar.activation(out=ex[:, :], in_=s_ps[:, :], func=AF.Exp,
                                         scale=1.0 / 16.0, accum_out=sm[:, :])
                    rs = watt.tile([128, 1], DT, tag="rs")
                    nc.vector.reciprocal(out=rs[:, :], in_=sm[:, :])
                    alpha = watt.tile([128, 128], BF, tag="alpha")
                    nc.vector.tensor_scalar(out=alpha[:, :], in0=ex[:, :],
                                            scalar1=rs[:, :1], scalar2=None,
                                            op0=ALU.mult)
                    # alpha [tt, ts] -> alphaT [ts, tt]
                    a_ps = pa.tile([128, 128], BF, tag="aps")
                    nc.tensor.transpose(a_ps[:, :], alpha[:, :], identb[:, :])
                    a_t = watt.tile([128, 128], BF, tag="at")
                    nc.scalar.copy(out=a_t[:, :], in_=a_ps[:, :])
                    # ctx'[d_m, tt] = He_T[ts, d_m].T @ alphaT[ts, tt]
                    for m in range(KC):
                        c_ps = pc.tile([128, 128], DT, tag="cps")
                        nc.tensor.matmul(out=c_ps[:, :], lhsT=het[:, b, m, :],
                                         rhs=a_t[:, :], start=True, stop=True)
                        if m == 0:
                            nc.vector.tensor_copy(out=ctx[:, m, b, :],
                                                  in_=c_ps[:, :])
                        else:
                            nc.scalar.copy(out=ctx[:, m, b, :], in_=c_ps[:, :])

            # ---- outs = hd + ctx @ ctx_W.T  -> houts[d, k, (b,t)] ----
            with tc.tile_pool(name="po", bufs=2, space="PSUM") as po:
                NBG = 8  # groups of 4 batches -> 512 psum cols
                for m in range(KC):
                    for g in range(NBG):
                        o_ps = po.tile([128, 512], DT, tag="ops")
                        for k in range(KC):
                            nc.tensor.matmul(
                                out=o_ps[:, :],
                                lhsT=cwt_sb[:, k, m * 128:(m + 1) * 128],
                                rhs=ctx[:, k, g * 4:(g + 1) * 4, :],
                                start=(k == 0), stop=(k == KC - 1))
                        nc.vector.tensor_add(
                            out=houts[:, m, g * 512:(g + 1) * 512],
                            in0=o_ps[:, :], in1=hd[:, m, g * 4:(g + 1) * 4, :])

            # =============== vocab projection ===============
            with tc.tile_pool(name="pl", bufs=4, space="PSUM") as pl:
                for b in range(B):
                    stg = sp.tile([128, VL], BF, tag="stg")
                    for vb in range(NVB):
                        l_ps = pl.tile([128, VB], DT, tag="lps")
                        for k in range(KC):
                            nc.tensor.matmul(
                                out=l_ps[:, :],
                                lhsT=houts[:, k, b * T:(b + 1) * T],
                                rhs=w_sb[:, k, vb * VB:(vb + 1) * VB],
                                start=(k == 0), stop=(k == KC - 1))
                        if vb % 2 == 0:
                            nc.vector.tensor_copy(
                                out=stg[:, vb * VB:(vb + 1) * VB], in_=l_ps[:, :])
                        else:
                            nc.scalar.copy(
                                out=stg[:, vb * VB:(vb + 1) * VB], in_=l_ps[:, :])
                    nc.sync.dma_start(out=out_d[b * T:(b + 1) * T, :],
                                      in_=stg[:, :])
    nc.compile()
    return nc


def _prep_in_maps(U, b_enc1, b_enc2, b_dec, E_en, E_de, ctx_W, W_out_de,
                  src_en, tgt_de_in):
    f32 = np.float32
    U = np.ascontiguousarray(U, f32).astype(NPBF)
    cwt = np.ascontiguousarray(np.asarray(ctx_W, f32).T).astype(NPBF)
    w_t = np.ascontiguousarray(np.asarray(W_out_de, f32).T)  # [D, V] f32
    E_en = np.asarray(E_en, f32)
    E_de = np.asarray(E_de, f32)
    src = np.asarray(src_en)
    tgt = np.asarray(tgt_de_in)

    def gather_x(E, idx, bias):
        # x'[p, k, b, t] = E[idx[t, b], k*128 + p] + bias[k*128+p]
        x = E[idx] + np.asarray(bias, f32)  # (T, B, D)
        x = x.transpose(2, 0, 1).reshape(KC, 128, T, B)  # (k, p, t, b)
        x = x.transpose(1, 2, 0, 3)  # (p, t, k, b)
        return np.ascontiguousarray(x).astype(NPBF)

    xs = gather_x(E_en, src, b_enc1)
    xt = gather_x(E_de, tgt, b_dec)
    b2row = np.asarray(b_enc2, f32).reshape(1, D).astype(NPBF)
    b2col = np.ascontiguousarray(np.asarray(b_enc2, f32).reshape(KC, 128).T)
    madd = np.where(src == 0, f32(-1e9), f32(0.0)).T.reshape(1, B * T)  # (b,t)
    madd = madd.astype(NPBF)

    in_maps = []
    for i in range(NCORES):
        in_maps.append({
            "u": U, "cwt": cwt,
            "w_slice": np.ascontiguousarray(
                w_t[:, i * VL:(i + 1) * VL]).astype(NPBF),
            "xs": xs, "xt": xt,
            "b2row": b2row, "b2col": b2col, "madd": madd,
        })
    return in_maps


def kernel(U, b_enc1, b_enc2, b_dec, E_en, E_de, ctx_W, W_out_de,
           src_en, tgt_de_in, _trace=False, _raw=False):
    if "nc" not in _CACHE:
        _CACHE["nc"] = _build()
    nc = _CACHE["nc"]
    in_maps = _prep_in_maps(U, b_enc1, b_enc2, b_dec, E_en, E_de, ctx_W,
                            W_out_de, src_en, tgt_de_in)
    res = run_bass_kernel_spmd(nc, in_maps, list(range(NCORES)), trace=_trace)
    if _raw:
        return res
    logits = np.empty((T, B, V), np.float32)
    for i in range(NCORES):
        blk = res.results[i]["out"].astype(np.float32).reshape(B, T, VL)
        logits[:, :, i * VL:(i + 1) * VL] = blk.transpose(1, 0, 2)
    if _trace:
        return logits, res
    return logits


# revision 7
# speedup vs baseline: 3.1286x; 1.0533x over previous
"""Seq2seq RNN with attention on 8 TRN2 NeuronCores.

Strategy v2:
- Host gathers embeddings (x = E[idx] + bias) and pre-transposes to the
  on-device layout, so the device never touches the 32000-row tables.
- Every core redundantly runs the full-batch (B=32) encoder and decoder
  *recurrences only* -- the per-step critical path is just
  PE(matmuls into PSUM, with x/bias injected via identity/ones matmuls)
  -> ACT(tanh).  No DVE on the critical path.
- Attention is computed *after* the decoder scan, batched over all
  timesteps (it does not feed back into the recurrence).
- The final vocab projection is tensor-parallel over V: core i computes
  columns [i*4000, (i+1)*4000) for the full batch, so each core loads
  only 2 MB of W and writes its 1/8 slice of the logits (bf16).
Output rows are (b, t)-major; the host reassembles (T, B, V) f32.
"""

import numpy as np

import concourse.bass as bass
import concourse.bacc as bacc
import concourse.tile as tile
from concourse import mybir
from concourse.bass_utils import run_bass_kernel_spmd
from concourse.masks import make_identity

D = 256
V = 32000
T = 128  # T_SRC == T_TGT == 128
B = 32
NCORES = 8
VL = V // NCORES  # 4000 vocab cols per core
KC = D // 128  # 2 d-chunks of 128
NVB = 8  # vocab col-blocks per core
VB = VL // NVB  # 500 cols per block (fits one PSUM bank in f32)
DT = mybir.dt.float32
BF = mybir.dt.bfloat16
NPBF = mybir.dt.np(BF)
AF = mybir.ActivationFunctionType
ALU = mybir.AluOpType

_CACHE = {}


def _build(with_b2):
    nc = bacc.Bacc(None)

    u_d = nc.declare_dram_parameter("u", [D, D], BF, isOutput=False)
    cwt_d = nc.declare_dram_parameter("cwt", [D, D], BF, isOutput=False)
    w_d = nc.declare_dram_parameter("w_slice", [D, VL], BF, isOutput=False)
    xs_d = nc.declare_dram_parameter("xs", [128, T, KC, B], BF, isOutput=False)
    xt_d = nc.declare_dram_parameter("xt", [128, T, KC, B], BF, isOutput=False)
    b2r_d = nc.declare_dram_parameter("b2row", [1, D], BF, isOutput=False)
    b2c_d = nc.declare_dram_parameter("b2col", [128, KC], DT, isOutput=False)
    madd_d = nc.declare_dram_parameter("madd", [1, B * T], BF, isOutput=False)
    out_d = nc.declare_dram_parameter("out", [B * T, VL], BF, isOutput=True)

    with tile.TileContext(nc) as tc:
        with (
            tc.tile_pool(name="persist", bufs=1) as pp,
            tc.tile_pool(name="stage", bufs=3) as sp,
        ):
            # ---- persistent SBUF tiles ----
            u_sb = pp.tile([128, KC, D], BF, tag="u")
            cwt_sb = pp.tile([128, KC, D], BF, tag="cwt")
            w_sb = pp.tile([128, KC, VL], BF, tag="w")
            ident = pp.tile([128, 128], DT, tag="ident")
            identb = pp.tile([128, 128], BF, tag="identb")
            ones_b = pp.tile([1, 128], BF, tag="ones")
            b2r_sb = pp.tile([1, D], BF, tag="b2r")
            b2c_sb = pp.tile([128, KC], DT, tag="b2c")
            madd_sb = pp.tile([1, B * T], BF, tag="madd")
            # x'/h' layouts: [d_lo, k, b, t]
            xs = pp.tile([128, T, KC, B], BF, tag="xs")
            xt = pp.tile([128, T, KC, B], BF, tag="xt")
            hd2 = pp.tile([128, 2, KC, B], BF, tag="hd2")  # enc l2 state (dbl-buf)
            hdc = pp.tile([128, 2, KC, B], BF, tag="hdc")  # dec state (dbl-buf)
            hd1 = pp.tile([128, KC, B], BF, tag="hd1")  # enc layer-1 state
            he = pp.tile([128, KC, B, T], BF, tag="he")  # enc H'
            hd = pp.tile([128, KC, B, T], BF, tag="hd")  # dec h'
            het = pp.tile([128, B, KC, 128], BF, tag="het")  # He_T[ts, b, k, d]
            ctx = pp.tile([128, KC, B, T], BF, tag="ctx")  # ctx'
            houts = pp.tile([128, KC, B * T], BF, tag="houts")  # outs'[d,k,(b,t)]

            # ---- load constants ----
            nc.sync.dma_start(out=xs[:, 0:16, :, :], in_=xs_d[:, 0:16, :, :])
            nc.sync.dma_start(out=xs[:, 16:, :, :], in_=xs_d[:, 16:, :, :])
            for k in range(KC):
                nc.sync.dma_start(out=u_sb[:, k, :], in_=u_d[k * 128:(k + 1) * 128, :])
                nc.sync.dma_start(out=cwt_sb[:, k, :],
                                  in_=cwt_d[k * 128:(k + 1) * 128, :])
            nc.sync.dma_start(out=b2r_sb[:, :], in_=b2r_d[:, :])
            nc.sync.dma_start(out=b2c_sb[:, :], in_=b2c_d[:, :])
            nc.sync.dma_start(out=madd_sb[:, :], in_=madd_d[:, :])
            nc.sync.dma_start(out=xt[:, :, :, :], in_=xt_d[:, :, :, :])
            # W slice streams in during the encoder scan
            for k in range(KC):
                nc.sync.dma_start(out=w_sb[:, k, :], in_=w_d[k * 128:(k + 1) * 128, :])
            make_identity(nc, ident[:, :])
            nc.vector.tensor_copy(out=identb[:, :], in_=ident[:, :])
            nc.gpsimd.memset(ones_b[:, :], 1.0)

            # =============== encoder scan ===============
            with (
                tc.tile_pool(name="pe1", bufs=2, space="PSUM") as pe1,
                tc.tile_pool(name="pe2", bufs=2, space="PSUM") as pe2,
            ):
                for t in range(T):
                    if t == 0:
                        # h1_0 = tanh(x_0)  (x includes b1)
                        nc.scalar.activation(out=hd1[:, :, :], in_=xs[:, 0, :, :],
                                             func=AF.Tanh)
                        # h2_0 = tanh(h1_0 + b2)
                        for m in range(KC):
                            nc.scalar.activation(out=hd2[:, 0, m, :],
                                                 in_=hd1[:, m, :], func=AF.Tanh,
                                                 bias=b2c_sb[:, m:m + 1])
                        nc.gpsimd.tensor_copy(out=he[:, :, :, 0],
                                              in_=hd2[:, 0, :, :])
                        continue
                    # ---- layer 1: p1 = x_t (+b1, folded) + U.T-chunks @ h1 ----
                    p1 = pe1.tile([128, KC, B], DT, tag="p1")
                    nc.tensor.matmul(out=p1[:, :, :], lhsT=identb[:, :],
                                     rhs=xs[:, t, :, :], start=True, stop=False,
                                     skip_group_check=True)
                    for m in range(KC):
                        for k in range(KC):
                            nc.tensor.matmul(
                                out=p1[:, m, :],
                                lhsT=u_sb[:, k, m * 128:(m + 1) * 128],
                                rhs=hd1[:, k, :],
                                start=False, stop=(k == KC - 1),
                                skip_group_check=True)
                    nc.scalar.activation(out=hd1[:, :, :], in_=p1[:, :, :],
                                         func=AF.Tanh)
                    # ---- layer 2: p2 = b2 + h1_t + U.T-chunks @ h2 ----
                    p2 = pe2.tile([128, KC, B], DT, tag="p2")
                    if with_b2:
                        for m in range(KC):
                            nc.tensor.matmul(out=p2[:, m, :],
                                             lhsT=b2r_sb[:, m * 128:(m + 1) * 128],
                                             rhs=ones_b[:, 0:B],
                                             start=(m == 0), stop=False,
                                             skip_group_check=True)
                    for m in range(KC):
                        for k in range(KC):
                            nc.tensor.matmul(
                                out=p2[:, m, :],
                                lhsT=u_sb[:, k, m * 128:(m + 1) * 128],
                                rhs=hd2[:, (t - 1) % 2, k, :],
                                start=(not with_b2 and m == 0 and k == 0),
                                stop=False,
                                skip_group_check=True)
                    nc.tensor.matmul(out=p2[:, :, :], lhsT=identb[:, :],
                                     rhs=hd1[:, :, :], start=False, stop=True,
                                     skip_group_check=True)
                    nc.scalar.activation(out=hd2[:, t % 2, :, :],
                                         in_=p2[:, :, :], func=AF.Tanh)
                    nc.gpsimd.tensor_copy(out=he[:, :, :, t],
                                          in_=hd2[:, t % 2, :, :])

            # =============== decoder scan (+ interleaved He transposes) ======
            with (
                tc.tile_pool(name="pd", bufs=2, space="PSUM") as pd,
                tc.tile_pool(name="pt", bufs=2, space="PSUM") as pt,
            ):
                tp_jobs = [(b, m) for b in range(B) for m in range(KC)]  # 64
                for t in range(T):
                    p = pd.tile([128, KC, B], DT, tag="pdec")
                    nc.tensor.matmul(out=p[:, :, :], lhsT=identb[:, :],
                                     rhs=xt[:, t, :, :], start=True, stop=False,
                                     skip_group_check=True)
                    for m in range(KC):
                        for k in range(KC):
                            prev = (hd2[:, (T - 1) % 2, k, :] if t == 0
                                    else hdc[:, (t - 1) % 2, k, :])
                            nc.tensor.matmul(
                                out=p[:, m, :],
                                lhsT=u_sb[:, k, m * 128:(m + 1) * 128],
                                rhs=prev,
                                start=False, stop=(k == KC - 1),
                                skip_group_check=True)
                    nc.scalar.activation(out=hdc[:, t % 2, :, :],
                                         in_=p[:, :, :], func=AF.Tanh)
                    nc.gpsimd.tensor_copy(out=hd[:, :, :, t],
                                          in_=hdc[:, t % 2, :, :])
                    # one He_T transpose every other step fills PE idle time;
                    # copies go on DVE only (ACT is on the scan critical path)
                    if t % 2 == 1 and tp_jobs:
                        b, m = tp_jobs.pop()
                        tps = pt.tile([128, 128], BF, tag="tps")
                        nc.tensor.transpose(tps[:, :], he[:, m, b, :], identb[:, :])
                        nc.vector.tensor_copy(out=het[:, b, m, :], in_=tps[:, :])

            # =============== attention (batched over t) ===============
            with (
                tc.tile_pool(name="ps", bufs=2, space="PSUM") as ps,
                tc.tile_pool(name="pa", bufs=2, space="PSUM") as pa,
                tc.tile_pool(name="pc", bufs=2, space="PSUM") as pc,
                tc.tile_pool(name="watt", bufs=4) as watt,
            ):
                for b in range(B):
                    # scores S[tt, ts] = mask + sum_d hd[d, tt] he[d, ts]
                    s_ps = ps.tile([128, 128], DT, tag="sps")
                    nc.tensor.matmul(out=s_ps[:, :], lhsT=ones_b[:, :],
                                     rhs=madd_sb[:, b * T:(b + 1) * T],
                                     start=True, stop=False,
                                     skip_group_check=True)
                    for k in range(KC):
                        nc.tensor.matmul(out=s_ps[:, :], lhsT=hd[:, k, b, :],
                                         rhs=he[:, k, b, :],
                                         start=False, stop=(k == KC - 1),
                                         skip_group_check=True)
                    # softmax over ts (free axis), scale 1/16 inside exp
                    ex = watt.tile([128, 128], BF, tag="ex")
                    sm = watt.tile([128, 1], DT, tag="sm")
                    nc.scalar.activation(out=ex[:, :], in_=s_ps[:, :], func=AF.Exp,
                                         scale=1.0 / 16.0, accum_out=sm[:, :])
                    rs = watt.tile([128, 1], DT, tag="rs")
                    nc.vector.reciprocal(out=rs[:, :], in_=sm[:, :])
                    alpha = watt.tile([128, 128], BF, tag="alpha")
                    nc.vector.tensor_scalar(out=alpha[:, :], in0=ex[:, :],
                                            scalar1=rs[:, :1], scalar2=None,
                                            op0=ALU.mult)
                    # alpha [tt, ts] -> alphaT [ts, tt]
                    a_ps = pa.tile([128, 128], BF, tag="aps")
                    nc.tensor.transpose(a_ps[:, :], alpha[:, :], identb[:, :])
                    a_t = watt.tile([128, 128], BF, tag="at")
                    nc.scalar.copy(out=a_t[:, :], in_=a_ps[:, :])
                    # ctx'[d_m, tt] = He_T[ts, d_m].T @ alphaT[ts, tt]
                    for m in range(KC):
                        c_ps = pc.tile([128, 128], DT, tag="cps")
                        nc.tensor.matmul(out=c_ps[:, :], lhsT=het[:, b, m, :],
                                         rhs=a_t[:, :], start=True, stop=True)
                        if m == 0:
                            nc.vector.tensor_copy(out=ctx[:, m, b, :],
                                                  in_=c_ps[:, :])
                        else:
                            nc.scalar.copy(out=ctx[:, m, b, :], in_=c_ps[:, :])

            # ---- outs = hd + ctx @ ctx_W.T  -> houts[d, k, (b,t)] ----
            with tc.tile_pool(name="po", bufs=2, space="PSUM") as po:
                NBG = 8  # groups of 4 batches -> 512 psum cols
                for m in range(KC):
                    for g in range(NBG):
                        o_ps = po.tile([128, 512], DT, tag="ops")
                        for k in range(KC):
                            nc.tensor.matmul(
                                out=o_ps[:, :],
                                lhsT=cwt_sb[:, k, m * 128:(m + 1) * 128],
                                rhs=ctx[:, k, g * 4:(g + 1) * 4, :],
                                start=(k == 0), stop=(k == KC - 1))
                        nc.vector.tensor_add(
                            out=houts[:, m, g * 512:(g + 1) * 512],
                            in0=o_ps[:, :], in1=hd[:, m, g * 4:(g + 1) * 4, :])

            # =============== vocab projection ===============
            with tc.tile_pool(name="pl", bufs=4, space="PSUM") as pl:
                for b in range(B):
                    stg = sp.tile([128, VL], BF, tag="stg")
                    for vb in range(NVB):
                        l_ps = pl.tile([128, VB], DT, tag="lps")
                        for k in range(KC):
                            nc.tensor.matmul(
                                out=l_ps[:, :],
                                lhsT=houts[:, k, b * T:(b + 1) * T],
                                rhs=w_sb[:, k, vb * VB:(vb + 1) * VB],
                                start=(k == 0), stop=(k == KC - 1))
                        if vb % 2 == 0:
                            nc.vector.tensor_copy(
                                out=stg[:, vb * VB:(vb + 1) * VB], in_=l_ps[:, :])
                        else:
                            nc.scalar.copy(
                                out=stg[:, vb * VB:(vb + 1) * VB], in_=l_ps[:, :])
                    nc.sync.dma_start(out=out_d[b * T:(b + 1) * T, :],
                                      in_=stg[:, :])
    nc.compile()
    return nc


def _prep_in_maps(U, b_enc1, b_enc2, b_dec, E_en, E_de, ctx_W, W_out_de,
                  src_en, tgt_de_in):
    f32 = np.float32
    U = np.ascontiguousarray(U, f32).astype(NPBF)
    cwt = np.ascontiguousarray(np.asarray(ctx_W, f32).T).astype(NPBF)
    w_t = np.ascontiguousarray(np.asarray(W_out_de, f32).T)  # [D, V] f32
    E_en = np.asarray(E_en, f32)
    E_de = np.asarray(E_de, f32)
    src = np.asarray(src_en)
    tgt = np.asarray(tgt_de_in)

    def gather_x(E, idx, bias):
        # x'[p, k, b, t] = E[idx[t, b], k*128 + p] + bias[k*128+p]
        x = E[idx] + np.asarray(bias, f32)  # (T, B, D)
        x = x.transpose(2, 0, 1).reshape(KC, 128, T, B)  # (k, p, t, b)
        x = x.transpose(1, 2, 0, 3)  # (p, t, k, b)
        return np.ascontiguousarray(x).astype(NPBF)

    xs = gather_x(E_en, src, b_enc1)
    xt = gather_x(E_de, tgt, b_dec)
    b2row = np.asarray(b_enc2, f32).reshape(1, D).astype(NPBF)
    b2col = np.ascontiguousarray(np.asarray(b_enc2, f32).reshape(KC, 128).T)
    madd = np.where(src == 0, f32(-1e9), f32(0.0)).T.reshape(1, B * T)  # (b,t)
    madd = madd.astype(NPBF)

    in_maps = []
    for i in range(NCORES):
        in_maps.append({
            "u": U, "cwt": cwt,
            "w_slice": np.ascontiguousarray(
                w_t[:, i * VL:(i + 1) * VL]).astype(NPBF),
            "xs": xs, "xt": xt,
            "b2row": b2row, "b2col": b2col, "madd": madd,
        })
    return in_maps


def kernel(U, b_enc1, b_enc2, b_dec, E_en, E_de, ctx_W, W_out_de,
           src_en, tgt_de_in, _trace=False, _raw=False):
    with_b2 = bool(np.any(np.asarray(b_enc2) != 0))
    key = ("nc", with_b2)
    if key not in _CACHE:
        _CACHE[key] = _build(with_b2)
    nc = _CACHE[key]
    in_maps = _prep_in_maps(U, b_enc1, b_enc2, b_dec, E_en, E_de, ctx_W,
                            W_out_de, src_en, tgt_de_in)
    res = run_bass_kernel_spmd(nc, in_maps, list(range(NCORES)), trace=_trace)
    if _raw:
        return res
    logits = np.empty((T, B, V), np.float32)
    for i in range(NCORES):
        blk = res.results[i]["out"].astype(np.float32).reshape(B, T, VL)
        logits[:, :, i * VL:(i + 1) * VL] = blk.transpose(1, 0, 2)
    if _trace:
        return logits, res
    return logits
